# revision 6
# baseline (speedup 1.0000x reference)
"""Trainium2 Bass kernel for nn_AttentionCell (MORAN attention cell + GRU).

Data-parallel over batch B across 8 NeuronCores. Each core processes a
B/8 = 32 batch slice:
  feats_ = feats @ w_i2h.T                        [T, b, H]   (big matmul)
  emit   = tanh(feats_ + (h @ w_h2h.T + b)) . w_score
  alpha  = softmax_T(emit)
  ctx    = sum_t alpha * feats
  nh     = GRUCell([ctx, cur_embed], h)

On-chip layout: batch-major row index m = b_local*T + t. Per group of 2
batches (512 rows): load feats naturally [128 t, 512 c], convert to bf16
(GPSIMD), PE-transpose to [128 c, 512 m], matmul against host-pre-
transposed w_i2h chunks producing feats_^T [128 h, 512 m] in PSUM, apply
tanh(+h_ bias per-partition) on ScalarE, dot with w_score on PE (M=1),
softmax per batch, and contract alpha against the natural-layout feats
tiles on PE for the context. GRU runs once at the end (tiny).

Weights are host-prepped (transposed / bias-folded / bf16) — activations
(feats, h, cur_embed) enter the device in their original layout/dtype.
"""

import numpy as np
import ml_dtypes
from contextlib import ExitStack

import concourse.bass as bass
import concourse.tile as tile
from concourse import bacc, mybir
from concourse.bass_utils import run_bass_kernel_spmd
from concourse.masks import make_identity

T, B, NIN = 256, 256, 512
NH, NEMB = 512, 256
NCORES = 8
BL = B // NCORES          # 32 local batches per core
NG = BL // 2              # 16 groups of 2 batches

F32 = mybir.dt.float32
BF = mybir.dt.bfloat16
AF = mybir.ActivationFunctionType
AX = mybir.AxisListType
BF_NP = ml_dtypes.bfloat16


def _build_program():
    nc = bacc.Bacc("TRN2", target_bir_lowering=False, debug=False)

    feats_d = nc.dram_tensor("feats", [T, BL, NIN], F32, kind="ExternalInput")
    h_d = nc.dram_tensor("h", [BL, NH], F32, kind="ExternalInput")
    ce_d = nc.dram_tensor("ce", [BL, NEMB], F32, kind="ExternalInput")
    wi2h_d = nc.dram_tensor("wi2hT", [NIN, NH], BF, kind="ExternalInput")
    wscore_d = nc.dram_tensor("wscoreT", [NH, 1], BF, kind="ExternalInput")
    wh2h_d = nc.dram_tensor("wh2hT", [NH, NH], BF, kind="ExternalInput")
    bh2h_d = nc.dram_tensor("bh2h", [1, NH], BF, kind="ExternalInput")
    wih_d = nc.dram_tensor("wihT", [NIN + NEMB, 3 * NH], BF, kind="ExternalInput")
    gibias_d = nc.dram_tensor("gibias", [1, 3 * NH], BF, kind="ExternalInput")
    whh_d = nc.dram_tensor("whhT", [NH, 3 * NH], BF, kind="ExternalInput")
    ghbias_d = nc.dram_tensor("ghbias", [1, 3 * NH], BF, kind="ExternalInput")
    out_nh_d = nc.dram_tensor("out_nh", [BL, NH], F32, kind="ExternalOutput")
    out_al_d = nc.dram_tensor("out_alpha", [T, BL], F32, kind="ExternalOutput")

    with tile.TileContext(nc) as tc, ExitStack() as ctx:
        const = ctx.enter_context(tc.tile_pool(name="const", bufs=1))

        ident_bf = const.tile([128, 128], BF)
        make_identity(nc, ident_bf)
        ident_f = const.tile([128, 128], F32)
        make_identity(nc, ident_f)
        ones_row = const.tile([1, 32], BF)
        nc.gpsimd.memset(ones_row, 1.0)

        # ---- weights to SBUF ----
        wi2h_sb = const.tile([128, 4 * NH], BF)      # [p, cc, h]
        nc.sync.dma_start(
            wi2h_sb.rearrange("p (k n) -> p k n", k=4),
            wi2h_d.rearrange("(k p) n -> p k n", p=128),
        )
        wscore_sb = const.tile([128, 4], BF)         # [p, hh]
        nc.sync.dma_start(
            wscore_sb.rearrange("p (k o) -> p k o", o=1),
            wscore_d.rearrange("(k p) o -> p k o", p=128),
        )
        wh2h_sb = const.tile([128, 4 * NH], BF)
        nc.sync.dma_start(
            wh2h_sb.rearrange("p (k n) -> p k n", k=4),
            wh2h_d.rearrange("(k p) n -> p k n", p=128),
        )
        wih_sb = const.tile([128, 6 * 3 * NH], BF)
        nc.sync.dma_start(
            wih_sb.rearrange("p (k n) -> p k n", k=6),
            wih_d.rearrange("(k p) n -> p k n", p=128),
        )
        whh_sb = const.tile([128, 4 * 3 * NH], BF)
        nc.sync.dma_start(
            whh_sb.rearrange("p (k n) -> p k n", k=4),
            whh_d.rearrange("(k p) n -> p k n", p=128),
        )
        bh2h_sb = const.tile([1, NH], BF)
        nc.sync.dma_start(bh2h_sb, bh2h_d[:, :])
        gibias_sb = const.tile([1, 3 * NH], BF)
        nc.sync.dma_start(gibias_sb, gibias_d[:, :])
        ghbias_sb = const.tile([1, 3 * NH], BF)
        nc.sync.dma_start(ghbias_sb, ghbias_d[:, :])

        # ---- persistent activations ----
        h_sb = const.tile([BL, NH], F32)
        nc.sync.dma_start(h_sb, h_d[:, :])
        ce_sb = const.tile([BL, NEMB], F32)
        nc.sync.dma_start(ce_sb, ce_d[:, :])

        hT_sb = const.tile([128, 4 * BL], BF)        # h^T for GRU matmuls
        hT_att = const.tile([128, 4 * BL], F32)      # (h@w_h2h.T + b)^T bias cols
        xT_sb = const.tile([128, 6 * BL], BF)        # [ctx^T; ce^T] for GRU
        alT_acc = const.tile([128, 2 * BL], F32)     # alpha^T accumulated [p, tc, b]
        ctx_nat = const.tile([BL, NIN], F32)         # context rows (natural)

        with tc.tile_pool(name="setup_ps", bufs=2, space="PSUM") as sps, \
             tc.tile_pool(name="setup_sb", bufs=2) as ssb:
            h_bf = ssb.tile([BL, NH], BF, tag="hbf")
            nc.vector.tensor_copy(h_bf, h_sb)
            for k in range(4):
                ps = sps.tile([128, BL], BF, tag="pst")
                nc.tensor.transpose(ps, h_bf[:, k * 128:(k + 1) * 128],
                                    ident_bf[:BL, :BL])
                nc.vector.tensor_copy(hT_sb[:, k * BL:(k + 1) * BL], ps)
            ce_bf = ssb.tile([BL, NEMB], BF, tag="cebf")
            nc.vector.tensor_copy(ce_bf, ce_sb)
            for k in range(2):
                ps = sps.tile([128, BL], BF, tag="pst")
                nc.tensor.transpose(ps, ce_bf[:, k * 128:(k + 1) * 128],
                                    ident_bf[:BL, :BL])
                nc.vector.tensor_copy(xT_sb[:, (4 + k) * BL:(5 + k) * BL], ps)
            # h_ = h @ w_h2h.T + b_h2h  (natural), then transpose -> hT_att
            psh = sps.tile([BL, NH], F32, tag="psh")
            for k in range(4):
                nc.tensor.matmul(psh, hT_sb[:, k * BL:(k + 1) * BL],
                                 wh2h_sb[:, k * NH:(k + 1) * NH],
                                 start=(k == 0), stop=False)
            nc.tensor.matmul(psh, ones_row, bh2h_sb, start=False, stop=True)
            h_nat = ssb.tile([BL, NH], F32, tag="hnat")
            nc.vector.tensor_copy(h_nat, psh)
            for k in range(4):
                ps = sps.tile([128, BL], F32, tag="pstf")
                nc.tensor.transpose(ps, h_nat[:, k * 128:(k + 1) * 128],
                                    ident_f[:BL, :BL])
                nc.vector.tensor_copy(hT_att[:, k * BL:(k + 1) * BL], ps)

        # ---- main loop over groups of 2 batches ----
        with tc.tile_pool(name="fnat", bufs=8) as fnat_p, \
             tc.tile_pool(name="fbf", bufs=8) as fbf_p, \
             tc.tile_pool(name="fT", bufs=8) as fT_p, \
             tc.tile_pool(name="tanh", bufs=8) as tanh_p, \
             tc.tile_pool(name="sm", bufs=4) as sm_p, \
             tc.tile_pool(name="albf", bufs=4) as albf_p, \
             tc.tile_pool(name="psT", bufs=2, space="PSUM") as psT_p, \
             tc.tile_pool(name="psM", bufs=2, space="PSUM") as psM_p, \
             tc.tile_pool(name="psE", bufs=2, space="PSUM") as psE_p, \
             tc.tile_pool(name="psA", bufs=2, space="PSUM") as psA_p:
            for g in range(NG):
                b0 = 2 * g
                # load + convert 4 natural tiles [128 t, 512 c]
                fbf = []
                for j in range(4):
                    b_loc, tch = divmod(j, 2)
                    f32t = fnat_p.tile([128, NIN], F32, tag="fnat")
                    nc.sync.dma_start(
                        f32t, feats_d[tch * 128:(tch + 1) * 128, b0 + b_loc, :])
                    bft = fbf_p.tile([128, NIN], BF, tag="fbf")
                    nc.gpsimd.tensor_copy(bft, f32t)
                    fbf.append(bft)
                # transpose to [128 c, 512 m]
                fT = []
                for cc in range(4):
                    t_sb = fT_p.tile([128, 512], BF, tag="fT")
                    for j in range(4):
                        ps = psT_p.tile([128, 128], BF, tag="psT")
                        nc.tensor.transpose(
                            ps, fbf[j][:, cc * 128:(cc + 1) * 128], ident_bf)
                        nc.vector.tensor_copy(
                            t_sb[:, j * 128:(j + 1) * 128], ps)
                    fT.append(t_sb)
                # feats_^T = w_i2h^T.T @ fT  -> [128 h, 512 m]; tanh; emit
                tanhT = []
                for hh in range(4):
                    psm = psM_p.tile([128, 512], F32, tag="psM")
                    for cc in range(4):
                        nc.tensor.matmul(
                            psm,
                            wi2h_sb[:, cc * NH + hh * 128: cc * NH + (hh + 1) * 128],
                            fT[cc], start=(cc == 0), stop=(cc == 3))
                    th = tanh_p.tile([128, 512], BF, tag="tanh")
                    for b_loc in range(2):
                        bias = hT_att[:, hh * BL + b0 + b_loc: hh * BL + b0 + b_loc + 1]
                        nc.scalar.activation(
                            th[:, b_loc * 256:(b_loc + 1) * 256],
                            psm[:, b_loc * 256:(b_loc + 1) * 256],
                            AF.Tanh, bias=bias)
                    tanhT.append(th)
                ps_em = psE_p.tile([1, 512], F32, tag="psE")
                for hh in range(4):
                    nc.tensor.matmul(ps_em, wscore_sb[:, hh:hh + 1], tanhT[hh],
                                     start=(hh == 0), stop=(hh == 3))
                # softmax over t for the 2 batches
                emrow = sm_p.tile([1, 512], F32, tag="emrow")
                nc.vector.tensor_copy(emrow, ps_em)
                emit2 = sm_p.tile([2, 256], F32, tag="emit2")
                nc.sync.dma_start(emit2[0:1, :], emrow[:, 0:256])
                nc.sync.dma_start(emit2[1:2, :], emrow[:, 256:512])
                negmax = sm_p.tile([2, 1], F32, tag="negmax")
                nc.vector.reduce_max(negmax, emit2, axis=AX.X, negate=True)
                expx = sm_p.tile([2, 256], F32, tag="expx")
                sums = sm_p.tile([2, 1], F32, tag="sums")
                nc.scalar.activation(expx, emit2, AF.Exp, bias=negmax,
                                     accum_out=sums)
                inv = sm_p.tile([2, 1], F32, tag="inv")
                nc.vector.reciprocal(inv, sums)
                alpha2 = sm_p.tile([2, 256], F32, tag="alpha2")
                nc.vector.tensor_scalar_mul(alpha2, expx, inv)
                # alpha^T: [128 t, 2 b] per t-chunk; f32 for output, bf16 for PE
                albf = albf_p.tile([128, 4], BF, tag="albf")
                for tch in range(2):
                    psa = psA_p.tile([128, 2], F32, tag="psA")
                    nc.tensor.transpose(
                        psa, alpha2[:, tch * 128:(tch + 1) * 128], ident_f[:2, :2])
                    nc.vector.tensor_copy(
                        alT_acc[:, tch * BL + b0: tch * BL + b0 + 2], psa)
                    nc.vector.tensor_copy(albf[:, tch * 2:(tch + 1) * 2], psa)
                # context rows via PE: alpha^T col (stationary) x fnat tiles
                for b_loc in range(2):
                    psc = psE_p.tile([1, 512], F32, tag="psE")
                    for tch in range(2):
                        nc.tensor.matmul(
                            psc, albf[:, tch * 2 + b_loc: tch * 2 + b_loc + 1],
                            fbf[b_loc * 2 + tch], start=(tch == 0), stop=(tch == 1))
                    crow = sm_p.tile([1, NIN], F32, tag="crow")
                    nc.vector.tensor_copy(crow, psc)
                    nc.sync.dma_start(ctx_nat[b0 + b_loc:b0 + b_loc + 1, :], crow)

        # ---- GRU tail ----
        with tc.tile_pool(name="gru_ps", bufs=1, space="PSUM") as gps, \
             tc.tile_pool(name="gru_sb", bufs=1) as gsb:
            ctx_bf = gsb.tile([BL, NIN], BF, tag="ctxbf")
            nc.vector.tensor_copy(ctx_bf, ctx_nat)
            for k in range(4):
                ps = gps.tile([128, BL], BF, tag="pst")
                nc.tensor.transpose(ps, ctx_bf[:, k * 128:(k + 1) * 128],
                                    ident_bf[:BL, :BL])
                nc.vector.tensor_copy(xT_sb[:, k * BL:(k + 1) * BL], ps)
            gi, gh = [], []
            for nn in range(3):
                pg = gps.tile([BL, NH], F32, tag=f"gi{nn}")
                for k in range(6):
                    nc.tensor.matmul(
                        pg, xT_sb[:, k * BL:(k + 1) * BL],
                        wih_sb[:, k * 3 * NH + nn * NH: k * 3 * NH + (nn + 1) * NH],
                        start=(k == 0), stop=False)
                nc.tensor.matmul(pg, ones_row,
                                 gibias_sb[:, nn * NH:(nn + 1) * NH],
                                 start=False, stop=True)
                gi.append(pg)
                ph = gps.tile([BL, NH], F32, tag=f"gh{nn}")
                for k in range(4):
                    nc.tensor.matmul(
                        ph, hT_sb[:, k * BL:(k + 1) * BL],
                        whh_sb[:, k * 3 * NH + nn * NH: k * 3 * NH + (nn + 1) * NH],
                        start=(k == 0), stop=False)
                nc.tensor.matmul(ph, ones_row,
                                 ghbias_sb[:, nn * NH:(nn + 1) * NH],
                                 start=False, stop=True)
                gh.append(ph)
            gh0_sb = gsb.tile([BL, NH], F32, tag="gh0sb")
            nc.scalar.copy(gh0_sb, gh[0])
            gh1_sb = gsb.tile([BL, NH], F32, tag="gh1sb")
            nc.scalar.copy(gh1_sb, gh[1])
            t0 = gsb.tile([BL, NH], F32, tag="t0")
            nc.vector.tensor_add(t0, gi[0], gh0_sb)
            r = gsb.tile([BL, NH], F32, tag="r")
            nc.scalar.activation(r, t0, AF.Sigmoid)
            t1 = gsb.tile([BL, NH], F32, tag="t1")
            nc.vector.tensor_add(t1, gi[1], gh1_sb)
            z = gsb.tile([BL, NH], F32, tag="z")
            nc.scalar.activation(z, t1, AF.Sigmoid)
            t2 = gsb.tile([BL, NH], F32, tag="t2")
            nc.vector.tensor_mul(t2, r, gh[2])
            t3 = gsb.tile([BL, NH], F32, tag="t3")
            nc.vector.tensor_add(t3, t2, gi[2])
            n_t = gsb.tile([BL, NH], F32, tag="n")
            nc.scalar.activation(n_t, t3, AF.Tanh)
            t4 = gsb.tile([BL, NH], F32, tag="t4")
            nc.vector.tensor_sub(t4, h_sb, n_t)
            t5 = gsb.tile([BL, NH], F32, tag="t5")
            nc.vector.tensor_mul(t5, z, t4)
            nh_t = gsb.tile([BL, NH], F32, tag="nh")
            nc.vector.tensor_add(nh_t, n_t, t5)
            nc.sync.dma_start(out_nh_d[:, :], nh_t)
            nc.sync.dma_start(
                out_al_d.rearrange("(k p) b -> p k b", p=128),
                alT_acc.rearrange("p (k b) -> p k b", k=2))

    nc.compile()
    return nc


_NC_CACHE = None


def _get_program():
    global _NC_CACHE
    if _NC_CACHE is None:
        _NC_CACHE = _build_program()
    return _NC_CACHE


def _prep_weights(inputs):
    f32 = lambda k: np.asarray(inputs[k], np.float32)
    bf = lambda a: np.ascontiguousarray(a, BF_NP)
    b_ih, b_hh = f32("b_ih"), f32("b_hh")
    gi_bias = b_ih + np.concatenate([b_hh[:NH], b_hh[NH:2 * NH], np.zeros(NH, np.float32)])
    gh_bias = np.concatenate([np.zeros(2 * NH, np.float32), b_hh[2 * NH:]])
    return {
        "wi2hT": bf(f32("w_i2h").T),
        "wscoreT": bf(f32("w_score").reshape(NH, 1)),
        "wh2hT": bf(f32("w_h2h").T),
        "bh2h": bf(f32("b_h2h").reshape(1, NH)),
        "wihT": bf(f32("w_ih").T),
        "gibias": bf(gi_bias.reshape(1, 3 * NH)),
        "whhT": bf(f32("w_hh").T),
        "ghbias": bf(gh_bias.reshape(1, 3 * NH)),
    }


def kernel(**inputs):
    feats = np.asarray(inputs["feats"], np.float32)
    h = np.asarray(inputs["h"], np.float32)
    ce = np.asarray(inputs["cur_embed"], np.float32)
    w = _prep_weights(inputs)
    nc = _get_program()
    in_maps = []
    for c in range(NCORES):
        sl = slice(c * BL, (c + 1) * BL)
        in_maps.append({
            "feats": np.ascontiguousarray(feats[:, sl, :]),
            "h": np.ascontiguousarray(h[sl]),
            "ce": np.ascontiguousarray(ce[sl]),
            **w,
        })
    res = run_bass_kernel_spmd(nc, in_maps, core_ids=list(range(NCORES)))
    nh = np.concatenate([res.results[c]["out_nh"] for c in range(NCORES)], axis=0)
    alpha = np.concatenate([res.results[c]["out_alpha"] for c in range(NCORES)], axis=1)
    return nh, alpha


# revision 9
# speedup vs baseline: 1.1827x; 1.1827x over previous
"""Trainium2 Bass kernel for nn_AttentionCell (MORAN attention cell + GRU).

Data-parallel over batch B across 8 NeuronCores; each core handles a
B/8 = 32 batch slice:
  feats_ = feats @ w_i2h.T                         (big matmul, bf16)
  emit   = tanh(feats_ + (h @ w_h2h.T + b)) . w_score
  alpha  = softmax_T(emit);  ctx = sum_t alpha * feats
  nh     = GRUCell([ctx, cur_embed], h)

Per group of 2 batches (512 rows, row index m = b_local*T + t):
  - feats loads f32 on the two HWDGE rings (sync/scalar), f32->bf16 on DVE
  - one XBAR dma_start_transpose produces feats^T [128 c', 2048]
  - TensorE: feats_^T [128 h', 512 m] = w_i2h^T chunks @ feats^T chunks
  - ScalarE: tanh(+ per-partition h_ bias), Exp for softmax
  - TensorE: emit row (w_score stationary), context rows (alpha stationary)
Softmax runs per group on a [2, 256] layout; alpha is transposed on the PE
for both the [T, BL] output and the context matmuls. GRU runs at the end.

Weights are host-prepped (transposed / bias-folded / bf16); activations
(feats, h, cur_embed) enter the device in their original layout/dtype.
"""

import numpy as np
import ml_dtypes
from contextlib import ExitStack

import concourse.bass as bass
import concourse.tile as tile
from concourse import bacc, mybir
from concourse.bass_utils import run_bass_kernel_spmd
from concourse.masks import make_identity

T, B, NIN = 256, 256, 512
NH, NEMB = 512, 256
NCORES = 8
BL = B // NCORES          # 32 local batches per core
NG = BL // 2              # 16 groups of 2 batches
NSG = NG // 4             # 4 super-groups of 8 batches

F32 = mybir.dt.float32
BF = mybir.dt.bfloat16
AF = mybir.ActivationFunctionType
AX = mybir.AxisListType
BF_NP = ml_dtypes.bfloat16


def _build_program():
    nc = bacc.Bacc("TRN2", target_bir_lowering=False, debug=False)

    feats_d = nc.dram_tensor("feats", [T, BL, NIN], F32, kind="ExternalInput")
    h_d = nc.dram_tensor("h", [BL, NH], F32, kind="ExternalInput")
    ce_d = nc.dram_tensor("ce", [BL, NEMB], F32, kind="ExternalInput")
    wi2h_d = nc.dram_tensor("wi2hT", [NIN, NH], BF, kind="ExternalInput")
    wscore_d = nc.dram_tensor("wscoreT", [NH, 1], BF, kind="ExternalInput")
    wh2h_d = nc.dram_tensor("wh2hT", [NH, NH], BF, kind="ExternalInput")
    bh2h_d = nc.dram_tensor("bh2h", [1, NH], BF, kind="ExternalInput")
    wih_d = nc.dram_tensor("wihT", [NIN + NEMB, 3 * NH], BF, kind="ExternalInput")
    gibias_d = nc.dram_tensor("gibias", [1, 3 * NH], BF, kind="ExternalInput")
    whh_d = nc.dram_tensor("whhT", [NH, 3 * NH], BF, kind="ExternalInput")
    ghbias_d = nc.dram_tensor("ghbias", [1, 3 * NH], BF, kind="ExternalInput")
    out_nh_d = nc.dram_tensor("out_nh", [BL, NH], F32, kind="ExternalOutput")
    out_al_d = nc.dram_tensor("out_alpha", [T, BL], F32, kind="ExternalOutput")

    with tile.TileContext(nc) as tc, ExitStack() as ctx:
        const = ctx.enter_context(tc.tile_pool(name="const", bufs=1))

        ident_bf = const.tile([128, 128], BF)
        make_identity(nc, ident_bf)
        ident_f = const.tile([128, 128], F32)
        make_identity(nc, ident_f)
        ones_row = const.tile([1, 32], BF)
        nc.gpsimd.memset(ones_row, 1.0)

        # ---- weights to SBUF ----
        wi2h_sb = const.tile([128, 4 * NH], BF)      # [p, cc, h]
        nc.sync.dma_start(
            wi2h_sb.rearrange("p (k n) -> p k n", k=4),
            wi2h_d.rearrange("(k p) n -> p k n", p=128),
        )
        wscore_sb = const.tile([128, 4], BF)         # [p, hh]
        nc.sync.dma_start(
            wscore_sb.rearrange("p (k o) -> p k o", o=1),
            wscore_d.rearrange("(k p) o -> p k o", p=128),
        )
        wh2h_sb = const.tile([128, 4 * NH], BF)
        nc.sync.dma_start(
            wh2h_sb.rearrange("p (k n) -> p k n", k=4),
            wh2h_d.rearrange("(k p) n -> p k n", p=128),
        )
        wih_sb = const.tile([128, 6 * 3 * NH], BF)
        nc.sync.dma_start(
            wih_sb.rearrange("p (k n) -> p k n", k=6),
            wih_d.rearrange("(k p) n -> p k n", p=128),
        )
        whh_sb = const.tile([128, 4 * 3 * NH], BF)
        nc.sync.dma_start(
            whh_sb.rearrange("p (k n) -> p k n", k=4),
            whh_d.rearrange("(k p) n -> p k n", p=128),
        )
        bh2h_sb = const.tile([1, NH], BF)
        nc.sync.dma_start(bh2h_sb, bh2h_d[:, :])
        gibias_sb = const.tile([1, 3 * NH], BF)
        nc.sync.dma_start(gibias_sb, gibias_d[:, :])
        ghbias_sb = const.tile([1, 3 * NH], BF)
        nc.sync.dma_start(ghbias_sb, ghbias_d[:, :])

        # ---- persistent activations ----
        h_sb = const.tile([BL, NH], F32)
        nc.sync.dma_start(h_sb, h_d[:, :])
        ce_sb = const.tile([BL, NEMB], F32)
        nc.sync.dma_start(ce_sb, ce_d[:, :])

        hT_sb = const.tile([128, 4 * BL], BF)        # h^T for GRU matmuls
        hT_att = const.tile([128, 4 * BL], F32)      # (h@w_h2h.T + b)^T bias cols
        xT_sb = const.tile([128, 6 * BL], BF)        # [ctx^T; ce^T] for GRU
        alT_acc = const.tile([128, 2 * BL], F32)     # alpha^T [p, tc, b]
        ctx_nat = const.tile([BL, NIN], F32)         # context rows (natural)

        with tc.tile_pool(name="setup_ps", bufs=2, space="PSUM") as sps, \
             tc.tile_pool(name="setup_sb", bufs=2) as ssb:
            h_bf = ssb.tile([BL, NH], BF, tag="hbf")
            nc.vector.tensor_copy(h_bf, h_sb)
            for k in range(4):
                ps = sps.tile([128, BL], BF, tag="pst")
                nc.tensor.transpose(ps, h_bf[:, k * 128:(k + 1) * 128],
                                    ident_bf[:BL, :BL])
                nc.vector.tensor_copy(hT_sb[:, k * BL:(k + 1) * BL], ps)
            ce_bf = ssb.tile([BL, NEMB], BF, tag="cebf")
            nc.vector.tensor_copy(ce_bf, ce_sb)
            for k in range(2):
                ps = sps.tile([128, BL], BF, tag="pst")
                nc.tensor.transpose(ps, ce_bf[:, k * 128:(k + 1) * 128],
                                    ident_bf[:BL, :BL])
                nc.vector.tensor_copy(xT_sb[:, (4 + k) * BL:(5 + k) * BL], ps)
            # h_ = h @ w_h2h.T + b_h2h  (natural), then transpose -> hT_att
            psh = sps.tile([BL, NH], F32, tag="psh")
            for k in range(4):
                nc.tensor.matmul(psh, hT_sb[:, k * BL:(k + 1) * BL],
                                 wh2h_sb[:, k * NH:(k + 1) * NH],
                                 start=(k == 0), stop=False)
            nc.tensor.matmul(psh, ones_row, bh2h_sb, start=False, stop=True)
            h_nat = ssb.tile([BL, NH], F32, tag="hnat")
            nc.vector.tensor_copy(h_nat, psh)
            for k in range(4):
                ps = sps.tile([128, BL], F32, tag="pstf")
                nc.tensor.transpose(ps, h_nat[:, k * 128:(k + 1) * 128],
                                    ident_f[:BL, :BL])
                nc.vector.tensor_copy(hT_att[:, k * BL:(k + 1) * BL], ps)

        # ---- main loop over 16 groups of 2 batches ----
        with tc.tile_pool(name="fnat", bufs=4) as fnat_p, \
             tc.tile_pool(name="fbf", bufs=4) as fbf_p, \
             tc.tile_pool(name="fT", bufs=3) as fT_p, \
             tc.tile_pool(name="tanh", bufs=8) as tanh_p, \
             tc.tile_pool(name="sm", bufs=4) as sm_p, \
             tc.tile_pool(name="albf", bufs=4) as albf_p, \
             tc.tile_pool(name="psM", bufs=3, space="PSUM") as psM_p, \
             tc.tile_pool(name="psE", bufs=2, space="PSUM") as psE_p, \
             tc.tile_pool(name="psA", bufs=2, space="PSUM") as psA_p:
            for g in range(NG):
                b0 = 2 * g
                # feats load f32 on the two HWDGE rings, convert on DVE,
                # layout [128 t', (b_loc, tch, c)]
                fnat = fnat_p.tile([128, 2048], F32, tag="fnat")
                fnat4 = fnat.rearrange("p (b tc c) -> p b tc c", b=2, tc=2)
                for tch in range(2):
                    eng = nc.sync if (g + tch) % 2 == 0 else nc.scalar
                    eng.dma_start(
                        fnat4[:, :, tch, :],
                        feats_d[tch * 128:(tch + 1) * 128, b0:b0 + 2, :])
                fbf = fbf_p.tile([128, 2048], BF, tag="fbf")
                nc.vector.tensor_copy(fbf, fnat)
                fbf4 = fbf.rearrange("p (b tc c) -> p b tc c", b=2, tc=2)
                # feats^T via DMA XBAR: fT[p, k, m] = fbf[m, 128k + p]
                fT = fT_p.tile([128, 2048], BF, tag="fT")
                eng = nc.sync if g % 2 == 0 else nc.scalar
                eng.dma_start_transpose(
                    fT.rearrange("p (k m) -> p k m", k=16), fbf)
                fT5 = fT.rearrange("p (b tc cc m) -> p b tc cc m",
                                   b=2, tc=2, cc=4)
                # feats_^T = w_i2h^T.T @ feats^T -> [128 h', 512 m]
                tanhT = []
                for hh in range(4):
                    psm = psM_p.tile([128, 512], F32, tag="psM")
                    for cc in range(4):
                        nc.tensor.matmul(
                            psm,
                            wi2h_sb[:, cc * NH + hh * 128:
                                    cc * NH + (hh + 1) * 128],
                            fT5[:, :, :, cc, :],
                            start=(cc == 0), stop=(cc == 3))
                    th = tanh_p.tile([128, 512], BF, tag="tanh")
                    for b_loc in range(2):
                        bias = hT_att[:, hh * BL + b0 + b_loc:
                                      hh * BL + b0 + b_loc + 1]
                        nc.scalar.activation(
                            th[:, b_loc * 256:(b_loc + 1) * 256],
                            psm[:, b_loc * 256:(b_loc + 1) * 256],
                            AF.Tanh, bias=bias)
                    tanhT.append(th)
                ps_em = psE_p.tile([1, 512], F32, tag="psE")
                for hh in range(4):
                    nc.tensor.matmul(ps_em, wscore_sb[:, hh:hh + 1],
                                     tanhT[hh],
                                     start=(hh == 0), stop=(hh == 3))
                # softmax over t for the 2 batches
                emrow = sm_p.tile([1, 512], F32, tag="emrow")
                nc.vector.tensor_copy(emrow, ps_em)
                emit2 = sm_p.tile([2, 256], F32, tag="emit2")
                nc.sync.dma_start(emit2[0:1, :], emrow[:, 0:256])
                nc.sync.dma_start(emit2[1:2, :], emrow[:, 256:512])
                negmax = sm_p.tile([2, 1], F32, tag="negmax")
                nc.vector.reduce_max(negmax, emit2, axis=AX.X, negate=True)
                expx = sm_p.tile([2, 256], F32, tag="expx")
                sums = sm_p.tile([2, 1], F32, tag="sums")
                nc.scalar.activation(expx, emit2, AF.Exp, bias=negmax,
                                     accum_out=sums)
                inv = sm_p.tile([2, 1], F32, tag="inv")
                nc.vector.reciprocal(inv, sums)
                alpha2 = sm_p.tile([2, 256], F32, tag="alpha2")
                nc.vector.tensor_scalar_mul(alpha2, expx, inv)
                # alpha^T: f32 for output accumulation, bf16 for PE
                albf = albf_p.tile([128, 4], BF, tag="albf")
                for tch in range(2):
                    psa = psA_p.tile([128, 2], F32, tag="psA")
                    nc.tensor.transpose(
                        psa, alpha2[:, tch * 128:(tch + 1) * 128],
                        ident_f[:2, :2])
                    nc.vector.tensor_copy(
                        alT_acc[:, tch * BL + b0: tch * BL + b0 + 2], psa)
                    nc.vector.tensor_copy(albf[:, tch * 2:(tch + 1) * 2], psa)
                # context rows via PE: alpha^T col stationary, feats moving
                for b_loc in range(2):
                    psc = psE_p.tile([1, NIN], F32, tag="psE")
                    for tch in range(2):
                        nc.tensor.matmul(
                            psc, albf[:, tch * 2 + b_loc: tch * 2 + b_loc + 1],
                            fbf4[:, b_loc, tch, :],
                            start=(tch == 0), stop=(tch == 1))
                    crow = sm_p.tile([1, NIN], F32, tag="crow")
                    nc.vector.tensor_copy(crow, psc)
                    nc.sync.dma_start(
                        ctx_nat[b0 + b_loc:b0 + b_loc + 1, :], crow)

        # ---- GRU tail ----
        with tc.tile_pool(name="gru_ps", bufs=1, space="PSUM") as gps, \
             tc.tile_pool(name="gru_sb", bufs=1) as gsb:
            ctx_bf = gsb.tile([BL, NIN], BF, tag="ctxbf")
            nc.vector.tensor_copy(ctx_bf, ctx_nat)
            for k in range(4):
                ps = gps.tile([128, BL], BF, tag="pst")
                nc.tensor.transpose(ps, ctx_bf[:, k * 128:(k + 1) * 128],
                                    ident_bf[:BL, :BL])
                nc.vector.tensor_copy(xT_sb[:, k * BL:(k + 1) * BL], ps)
            gi_l, gh_l = [], []
            for nn in range(3):
                pg = gps.tile([BL, NH], F32, tag=f"gi{nn}")
                for k in range(6):
                    nc.tensor.matmul(
                        pg, xT_sb[:, k * BL:(k + 1) * BL],
                        wih_sb[:, k * 3 * NH + nn * NH:
                               k * 3 * NH + (nn + 1) * NH],
                        start=(k == 0), stop=False)
                nc.tensor.matmul(pg, ones_row,
                                 gibias_sb[:, nn * NH:(nn + 1) * NH],
                                 start=False, stop=True)
                gi_l.append(pg)
                ph = gps.tile([BL, NH], F32, tag=f"gh{nn}")
                for k in range(4):
                    nc.tensor.matmul(
                        ph, hT_sb[:, k * BL:(k + 1) * BL],
                        whh_sb[:, k * 3 * NH + nn * NH:
                               k * 3 * NH + (nn + 1) * NH],
                        start=(k == 0), stop=False)
                nc.tensor.matmul(ph, ones_row,
                                 ghbias_sb[:, nn * NH:(nn + 1) * NH],
                                 start=False, stop=True)
                gh_l.append(ph)
            gh0_sb = gsb.tile([BL, NH], F32, tag="gh0sb")
            nc.scalar.copy(gh0_sb, gh_l[0])
            gh1_sb = gsb.tile([BL, NH], F32, tag="gh1sb")
            nc.scalar.copy(gh1_sb, gh_l[1])
            t0 = gsb.tile([BL, NH], F32, tag="t0")
            nc.vector.tensor_add(t0, gi_l[0], gh0_sb)
            r = gsb.tile([BL, NH], F32, tag="r")
            nc.scalar.activation(r, t0, AF.Sigmoid)
            t1 = gsb.tile([BL, NH], F32, tag="t1")
            nc.vector.tensor_add(t1, gi_l[1], gh1_sb)
            z = gsb.tile([BL, NH], F32, tag="z")
            nc.scalar.activation(z, t1, AF.Sigmoid)
            t2 = gsb.tile([BL, NH], F32, tag="t2")
            nc.vector.tensor_mul(t2, r, gh_l[2])
            t3 = gsb.tile([BL, NH], F32, tag="t3")
            nc.vector.tensor_add(t3, t2, gi_l[2])
            n_t = gsb.tile([BL, NH], F32, tag="n")
            nc.scalar.activation(n_t, t3, AF.Tanh)
            t4 = gsb.tile([BL, NH], F32, tag="t4")
            nc.vector.tensor_sub(t4, h_sb, n_t)
            t5 = gsb.tile([BL, NH], F32, tag="t5")
            nc.vector.tensor_mul(t5, z, t4)
            nh_t = gsb.tile([BL, NH], F32, tag="nh")
            nc.vector.tensor_add(nh_t, n_t, t5)
            nc.sync.dma_start(out_nh_d[:, :], nh_t)
            nc.sync.dma_start(
                out_al_d.rearrange("(k p) b -> p k b", p=128),
                alT_acc.rearrange("p (k b) -> p k b", k=2))

    nc.compile()
    return nc


_NC_CACHE = None


def _get_program():
    global _NC_CACHE
    if _NC_CACHE is None:
        _NC_CACHE = _build_program()
    return _NC_CACHE


def _prep_weights(inputs):
    f32 = lambda k: np.asarray(inputs[k], np.float32)
    bf = lambda a: np.ascontiguousarray(a, BF_NP)
    b_ih, b_hh = f32("b_ih"), f32("b_hh")
    gi_bias = b_ih + np.concatenate([b_hh[:NH], b_hh[NH:2 * NH], np.zeros(NH, np.float32)])
    gh_bias = np.concatenate([np.zeros(2 * NH, np.float32), b_hh[2 * NH:]])
    return {
        "wi2hT": bf(f32("w_i2h").T),
        "wscoreT": bf(f32("w_score").reshape(NH, 1)),
        "wh2hT": bf(f32("w_h2h").T),
        "bh2h": bf(f32("b_h2h").reshape(1, NH)),
        "wihT": bf(f32("w_ih").T),
        "gibias": bf(gi_bias.reshape(1, 3 * NH)),
        "whhT": bf(f32("w_hh").T),
        "ghbias": bf(gh_bias.reshape(1, 3 * NH)),
    }


def kernel(**inputs):
    feats = np.asarray(inputs["feats"], np.float32)
    h = np.asarray(inputs["h"], np.float32)
    ce = np.asarray(inputs["cur_embed"], np.float32)
    w = _prep_weights(inputs)
    nc = _get_program()
    in_maps = []
    for c in range(NCORES):
        sl = slice(c * BL, (c + 1) * BL)
        in_maps.append({
            "feats": np.ascontiguousarray(feats[:, sl, :]),
            "h": np.ascontiguousarray(h[sl]),
            "ce": np.ascontiguousarray(ce[sl]),
            **w,
        })
    res = run_bass_kernel_spmd(nc, in_maps, core_ids=list(range(NCORES)))
    nh = np.concatenate([res.results[c]["out_nh"] for c in range(NCORES)], axis=0)
    alpha = np.concatenate([res.results[c]["out_alpha"] for c in range(NCORES)], axis=1)
    return nh, alpha
